# revision 1
# baseline (speedup 1.0000x reference)
"""Trainium2 Bass kernel for CostAwareHeteroMoE.

Strategy: data-parallel over tokens across 8 NeuronCores (1024 tokens/core),
all weights replicated, no collectives. Activations are kept feature-major
([features-on-partitions, tokens-on-free]) so every matmul chains without
transposes; per-token routing weights are applied along the free dim via a
broadcast tile built on-chip.

Math rewrite (validated vs reference at ~3e-7 rel err):
  out = (sum_e W[:,e] * (gelu(gelu(h) @ w1_e + b1_e) @ w2_e + b2'_e)
         + 0.1 * gelu(gelu(h) @ sw1 + sb1) @ sw2 + const) @ up_w + up_b'
        + gelu(x) @ core_w + core_b
where W[:,e] are dense top-2 routing weights (0 elsewhere), b2'_e = b2_e - c_e
folds the "bias leak" of unrouted tokens (c_e = gelu(b1_e) @ w2_e + b2_e,
which reference adds for every unrouted token), and up_b' absorbs the
constant  (sum_e c_e + 0.1 * sb2) @ up_w.
"""

import sys

import numpy as np

sys.path.insert(0, "/opt/trn_rl_repo")

B, T, D, L = 4, 2048, 2048, 1024
HID = [1024, 2048, 3072, 4096, 1024, 2048, 3072, 4096]
E = 8
TOP_K = 2
COST_LAMBDA = 1e-7
NCORES = 8
NTOK = B * T
TPC = NTOK // NCORES  # 1024 tokens per core
P = 128
HGRP = 512  # expert hidden rows per weight-slice group


def _gelu_np(v):
    from scipy.special import erf

    return 0.5 * v * (1.0 + erf(v / np.sqrt(2.0)))


def _build_program():
    import concourse.bass as bass
    from concourse import bacc
    import concourse.mybir as mybir
    import concourse.tile as tile
    from concourse.masks import make_identity

    f32 = mybir.dt.float32
    f32r = mybir.dt.float32r
    AF = mybir.ActivationFunctionType
    ALU = mybir.AluOpType
    AX = mybir.AxisListType

    def r(ap):  # operands are already fp32r-typed
        return ap

    nc = bacc.Bacc("TRN2", debug=False)

    # ---- DRAM I/O ----
    xt = nc.dram_tensor("xt", [D, TPC], f32r, kind="ExternalInput").ap()
    dw = nc.dram_tensor("dw", [D, L], f32r, kind="ExternalInput").ap()
    rw = nc.dram_tensor("rw", [D, E], f32r, kind="ExternalInput").ap()
    upw = nc.dram_tensor("upw", [L, D], f32r, kind="ExternalInput").ap()
    corw = nc.dram_tensor("corw", [D, D], f32r, kind="ExternalInput").ap()
    sw1 = nc.dram_tensor("sw1", [L, L], f32r, kind="ExternalInput").ap()
    sw2 = nc.dram_tensor("sw2", [L, L], f32r, kind="ExternalInput").ap()  # pre-scaled by 0.1
    ew1 = [nc.dram_tensor(f"e{e}w1", [L, HID[e]], f32r, kind="ExternalInput").ap() for e in range(E)]
    ew2 = [nc.dram_tensor(f"e{e}w2", [HID[e], L], f32r, kind="ExternalInput").ap() for e in range(E)]
    # biases, host-prelaid into per-partition layouts
    dbpp = nc.dram_tensor("dbpp", [P, L // P], f32, kind="ExternalInput").ap()
    sb1pp = nc.dram_tensor("sb1pp", [P, L // P], f32, kind="ExternalInput").ap()
    b1pp = [nc.dram_tensor(f"b1pp{e}", [P, HID[e] // P], f32, kind="ExternalInput").ap() for e in range(E)]
    b2mat = nc.dram_tensor("b2mat", [E, L], f32r, kind="ExternalInput").ap()
    obias = nc.dram_tensor("obias", [1, D], f32r, kind="ExternalInput").ap()
    rbias = nc.dram_tensor("rbias", [1, E], f32r, kind="ExternalInput").ap()
    onesv = nc.dram_tensor("onesv", [1, 512], f32r, kind="ExternalInput").ap()
    out = nc.dram_tensor("out", [D, TPC], f32, kind="ExternalOutput").ap()
    wt_dram = nc.dram_tensor("wt_scratch", [E, TPC], f32r).ap()  # internal scratch

    KD = D // P      # 16 k-chunks over D
    KL = L // P      # 8 k-chunks over L
    NH = TPC // 512  # 2 token halves of 512

    with tile.TileContext(nc) as tc:
        import contextlib

        with contextlib.ExitStack() as ctx:
            const = ctx.enter_context(tc.tile_pool(name="const", bufs=1))
            gpool = ctx.enter_context(tc.tile_pool(name="gpool", bufs=1))
            psum = ctx.enter_context(tc.tile_pool(name="psum", bufs=2, space="PSUM"))

            ident = const.tile([P, P], f32)
            make_identity(nc, ident)
            ones = const.tile([1, 512], f32r)
            nc.sync.dma_start(ones, onesv)

            rw_sb = const.tile([P, KD, E], f32r)
            nc.sync.dma_start(rw_sb, rw.rearrange("(ko ki) e -> ki ko e", ki=P))
            rb_sb = const.tile([1, E], f32r)
            nc.sync.dma_start(rb_sb, rbias)
            db_sb = const.tile([P, L // P], f32)
            nc.sync.dma_start(db_sb, dbpp)
            sb1_sb = const.tile([P, L // P], f32)
            nc.sync.dma_start(sb1_sb, sb1pp)
            b1_sb = []
            for e in range(E):
                t_ = const.tile([P, HID[e] // P], f32, tag=f"b1sb{e}")
                nc.sync.dma_start(t_, b1pp[e])
                b1_sb.append(t_)
            b2_sb = const.tile([E, L], f32r)
            nc.sync.dma_start(b2_sb, b2mat)
            ob_sb = const.tile([1, D], f32r)
            nc.sync.dma_start(ob_sb, obias)
            wt_sb = const.tile([E, TPC], f32r)  # routing weights, feature-major [E, tokens]

            g = gpool.tile([P, KL, TPC], f32r)   # gelu(h), feature-major
            y = gpool.tile([P, KL, TPC], f32r)   # pre-up accumulator (first written by b2 pass)

            xt3 = xt.rearrange("(ko ki) t -> ki ko t", ki=P)

            # ============ Stage A+B: router + down-projection ============
            with contextlib.ExitStack() as sab:
                xpool = sab.enter_context(tc.tile_pool(name="xpool", bufs=2))
                dwp = sab.enter_context(tc.tile_pool(name="dwp", bufs=2))
                rwork = sab.enter_context(tc.tile_pool(name="rwork", bufs=4))
                rpsum = sab.enter_context(tc.tile_pool(name="rpsum", bufs=2, space="PSUM"))
                tpsum = sab.enter_context(tc.tile_pool(name="tpsum", bufs=2, space="PSUM"))

                for half in range(NH):
                    ts_ = slice(half * 512, (half + 1) * 512)
                    xth = xpool.tile([P, KD, 512], f32r, tag="xth")
                    nc.sync.dma_start(xth, xt3[:, :, ts_])

                    # ---- router on this half's 4 token-chunks of 128 ----
                    for tj in range(4):
                        t0 = half * 512 + tj * 128
                        rp = rpsum.tile([P, E], f32, tag="rp")
                        for k in range(KD):
                            nc.tensor.matmul(
                                rp, r(xth[:, k, tj * 128:(tj + 1) * 128]), r(rw_sb[:, k, :]),
                                start=(k == 0), stop=False,
                            )
                        nc.tensor.matmul(rp, r(ones[:, :P]), r(rb_sb), start=False, stop=True)
                        nmax = rwork.tile([P, 1], f32, tag="nmax")
                        nc.vector.tensor_reduce(nmax, rp, axis=AX.X, op=ALU.max, negate=True)
                        pexp = rwork.tile([P, E], f32, tag="pexp")
                        nc.scalar.activation(pexp, rp, AF.Exp, bias=nmax)
                        ssum = rwork.tile([P, 1], f32, tag="ssum")
                        nc.vector.tensor_reduce(ssum, pexp, axis=AX.X, op=ALU.add)
                        rs = rwork.tile([P, 1], f32, tag="rs")
                        nc.vector.reciprocal(rs, ssum)
                        probs = rwork.tile([P, E], f32, tag="probs")
                        nc.vector.tensor_scalar_mul(probs, pexp, rs)
                        p1 = rwork.tile([P, 1], f32, tag="p1")
                        nc.vector.tensor_reduce(p1, probs, axis=AX.X, op=ALU.max)
                        mlt = rwork.tile([P, E], f32, tag="mlt")
                        nc.vector.tensor_scalar(mlt, probs, p1, None, op0=ALU.is_lt)
                        pz = rwork.tile([P, E], f32, tag="pz")
                        nc.vector.tensor_mul(pz, probs, mlt)
                        p2 = rwork.tile([P, 1], f32, tag="p2")
                        nc.vector.tensor_reduce(p2, pz, axis=AX.X, op=ALU.max)
                        dd = rwork.tile([P, 1], f32, tag="dd")
                        nc.vector.tensor_scalar(dd, p2, p1, None, op0=ALU.subtract)
                        s2 = rwork.tile([P, 1], f32, tag="s2")
                        nc.scalar.activation(s2, dd, AF.Sigmoid)
                        s1 = rwork.tile([P, 1], f32, tag="s1")
                        nc.vector.tensor_scalar(s1, s2, -1.0, 1.0, op0=ALU.mult, op1=ALU.add)
                        m1 = rwork.tile([P, E], f32, tag="m1")
                        nc.vector.tensor_scalar(m1, probs, p1, None, op0=ALU.is_ge)
                        m2 = rwork.tile([P, E], f32, tag="m2")
                        nc.vector.tensor_scalar(m2, pz, p2, None, op0=ALU.is_ge)
                        wc1 = rwork.tile([P, E], f32, tag="wc1")
                        nc.vector.tensor_scalar_mul(wc1, m1, s1)
                        wc = rwork.tile([P, E], f32, tag="wc")
                        nc.vector.tensor_scalar_mul(wc, m2, s2)
                        nc.vector.tensor_add(wc, wc, wc1)
                        # transpose [128 tok, E] -> [E, 128 tok] into wt_sb
                        tp = tpsum.tile([E, P], f32, tag="tp")
                        nc.tensor.transpose(tp, wc, ident)
                        nc.vector.tensor_copy(wt_sb[:, t0:t0 + 128], tp)
                        nc.sync.dma_start(wt_dram[:, t0:t0 + 128], wt_sb[:, t0:t0 + 128])

                    # ---- down-projection for this half ----
                    for m in range(KL):
                        dsl = dwp.tile([P, KD, P], f32r, tag="dsl")
                        nc.sync.dma_start(
                            dsl, dw.rearrange("(ko ki) l -> ki ko l", ki=P)[:, :, m * P:(m + 1) * P]
                        )
                        hp = psum.tile([P, 512], f32, tag="a")
                        for k in range(KD):
                            nc.tensor.matmul(
                                hp, r(dsl[:, k, :]), r(xth[:, k, :]),
                                start=(k == 0), stop=(k == KD - 1),
                            )
                        nc.scalar.activation(g[:, m, ts_], hp, AF.Gelu, bias=db_sb[:, m:m + 1])

            # ============ Stage C: experts (+ shared, + b2 correction) ============
            with contextlib.ExitStack() as sex:
                wbp = sex.enter_context(tc.tile_pool(name="wbp", bufs=1))
                wep = sex.enter_context(tc.tile_pool(name="wep", bufs=2))
                ework = sex.enter_context(tc.tile_pool(name="ework", bufs=8))
                gawork = sex.enter_context(tc.tile_pool(name="gawork", bufs=3))

                # broadcast routing weights to all partitions: Wb[p, e, t] = W[t, e]
                wb = wbp.tile([P, E, TPC], f32r)
                nc.sync.dma_start(wb, wt_dram.partition_broadcast(P))

                # b2' correction initializes y: y = W @ b2mat   (K=E matmul)
                for m in range(KL):
                    for half in range(NH):
                        ts_ = slice(half * 512, (half + 1) * 512)
                        yp = psum.tile([P, 512], f32, tag="y")
                        nc.tensor.matmul(
                            yp, r(b2_sb[:, m * P:(m + 1) * P]), r(wt_sb[:, ts_]), start=True, stop=True
                        )
                        nc.vector.tensor_copy(y[:, m, ts_], yp)

                def mlp_block(w1_ap, w2_ap, h_dim, b1_tile, scale_e):
                    """y += [Wb_e *] gelu(w1.T@g + b1) via w2, streamed in HGRP row groups."""
                    for gi in range(h_dim // HGRP):
                        w1s = wep.tile([P, KL, HGRP], f32r, tag="w1s")
                        nc.sync.dma_start(
                            w1s,
                            w1_ap.rearrange("(ko ki) h -> ki ko h", ki=P)[:, :, gi * HGRP:(gi + 1) * HGRP],
                        )
                        w2s = wep.tile([P, HGRP // P, L], f32r, tag="w2s")
                        nc.sync.dma_start(
                            w2s,
                            w2_ap.rearrange("(ko ki) l -> ki ko l", ki=P)[:, gi * (HGRP // P):(gi + 1) * (HGRP // P), :],
                        )
                        for half in range(NH):
                            ts_ = slice(half * 512, (half + 1) * 512)
                            sga = []
                            for hc in range(HGRP // P):
                                ap_ = psum.tile([P, 512], f32, tag="a")
                                for k in range(KL):
                                    nc.tensor.matmul(
                                        ap_, r(w1s[:, k, hc * P:(hc + 1) * P]), r(g[:, k, ts_]),
                                        start=(k == 0), stop=(k == KL - 1),
                                    )
                                ga = gawork.tile([P, 512], f32r, tag="ga")
                                nc.scalar.activation(
                                    ga, ap_, AF.Gelu,
                                    bias=b1_tile[:, gi * (HGRP // P) + hc: gi * (HGRP // P) + hc + 1],
                                )
                                sg = ework.tile([P, 512], f32r, tag="sga")
                                if scale_e is not None:
                                    nc.vector.tensor_mul(sg, ga, wb[:, scale_e, ts_])
                                else:
                                    nc.vector.tensor_copy(sg, ga)
                                sga.append(sg)
                            for m in range(KL):
                                yp = psum.tile([P, 512], f32, tag="y")
                                for hc in range(HGRP // P):
                                    nc.tensor.matmul(
                                        yp, r(w2s[:, hc, m * P:(m + 1) * P]), r(sga[hc]),
                                        start=(hc == 0), stop=(hc == HGRP // P - 1),
                                    )
                                nc.vector.tensor_add(y[:, m, ts_], y[:, m, ts_], yp)

                for e in range(E):
                    mlp_block(ew1[e], ew2[e], HID[e], b1_sb[e], e)
                mlp_block(sw1, sw2, L, sb1_sb, None)  # shared branch (w2 pre-scaled 0.1)

            # ============ Stage E: up-projection + core branch ============
            with contextlib.ExitStack() as se:
                gxp = se.enter_context(tc.tile_pool(name="gxp", bufs=1))
                stg = se.enter_context(tc.tile_pool(name="stg", bufs=2))
                wup = se.enter_context(tc.tile_pool(name="wup", bufs=2))
                otp = se.enter_context(tc.tile_pool(name="otp", bufs=3))

                gx = gxp.tile([P, KD, TPC], f32r)
                for k in range(KD):
                    st_ = stg.tile([P, TPC], f32r, tag="st")
                    nc.sync.dma_start(st_, xt3[:, k, :])
                    nc.scalar.activation(gx[:, k, :], st_, AF.Gelu)

                for m in range(KD):
                    ms = slice(m * P, (m + 1) * P)
                    usl = wup.tile([P, KL, P], f32r, tag="usl")
                    nc.sync.dma_start(usl, upw.rearrange("(ko ki) d -> ki ko d", ki=P)[:, :, ms])
                    csl = wup.tile([P, KD, P], f32r, tag="csl")
                    nc.sync.dma_start(csl, corw.rearrange("(ko ki) d -> ki ko d", ki=P)[:, :, ms])
                    for half in range(NH):
                        ts_ = slice(half * 512, (half + 1) * 512)
                        op_ = psum.tile([P, 512], f32, tag="a")
                        for k in range(KL):
                            nc.tensor.matmul(op_, r(usl[:, k, :]), r(y[:, k, ts_]), start=(k == 0), stop=False)
                        for k in range(KD):
                            nc.tensor.matmul(op_, r(csl[:, k, :]), r(gx[:, k, ts_]), start=False, stop=False)
                        nc.tensor.matmul(op_, r(ob_sb[:1, ms]), r(ones[:1, :512]), start=False, stop=True)
                        ot = otp.tile([P, 512], f32, tag="ot")
                        nc.vector.tensor_copy(ot, op_)
                        nc.sync.dma_start(out[ms, ts_], ot)

    nc.finalize()
    return nc


def kernel(**inputs):
    from concourse.bass_utils import run_bass_kernel_spmd

    inp = {k: np.ascontiguousarray(np.asarray(v, dtype=np.float32)) for k, v in inputs.items()}
    x = inp["x"].reshape(NTOK, D)

    # ---- host-side weight preprocessing (pure layout/folding, no token math) ----
    cost = np.array([2 * L * h for h in HID], np.float32)
    rbias = (inp["router_b"] - COST_LAMBDA * cost).reshape(1, E)
    c = [
        _gelu_np(inp[f"e{e}_b1"]) @ inp[f"e{e}_w2"] + inp[f"e{e}_b2"]
        for e in range(E)
    ]
    b2mat = np.stack([inp[f"e{e}_b2"] - c[e] for e in range(E)], axis=0)  # [E, L]
    const_l = np.sum(c, axis=0) + 0.1 * inp["shared_b2"]
    obias = (inp["up_b"] + const_l @ inp["up_w"] + inp["core_b"]).reshape(1, D)

    common = {
        "dw": inp["down_w"],
        "rw": inp["router_w"],
        "upw": inp["up_w"],
        "corw": inp["core_w"],
        "sw1": inp["shared_w1"],
        "sw2": np.ascontiguousarray(0.1 * inp["shared_w2"]),
        "dbpp": np.ascontiguousarray(inp["down_b"].reshape(L // P, P).T),
        "sb1pp": np.ascontiguousarray(inp["shared_b1"].reshape(L // P, P).T),
        "b2mat": np.ascontiguousarray(b2mat),
        "obias": np.ascontiguousarray(obias),
        "rbias": np.ascontiguousarray(rbias),
        "onesv": np.ones((1, 512), np.float32),
    }
    for e in range(E):
        common[f"e{e}w1"] = inp[f"e{e}_w1"]
        common[f"e{e}w2"] = inp[f"e{e}_w2"]
        common[f"b1pp{e}"] = np.ascontiguousarray(inp[f"e{e}_b1"].reshape(HID[e] // P, P).T)

    in_maps = []
    for cidx in range(NCORES):
        m = dict(common)
        m["xt"] = np.ascontiguousarray(x[cidx * TPC:(cidx + 1) * TPC].T)
        in_maps.append(m)

    nc = _build_program()
    res = run_bass_kernel_spmd(nc, in_maps, list(range(NCORES)))

    full = np.empty((NTOK, D), np.float32)
    for cidx in range(NCORES):
        full[cidx * TPC:(cidx + 1) * TPC] = res.results[cidx]["out"].T
    return full.reshape(B, T, D)



# revision 20
# speedup vs baseline: 1.1017x; 1.1017x over previous
"""Trainium2 Bass kernel for CostAwareHeteroMoE — sparse expert dispatch.

Strategy: data-parallel over tokens across 8 NeuronCores (1024 tokens/core).
Unlike the dense-masked v1 (which ran all 8 experts on all tokens), this
version computes each token only through its top-2 experts:

  - Host: fp32 routing is replicated on host ONLY to pick per-expert
    capacity constants (compile-time shapes) and to balance the token->core
    assignment. All token math (router, top-2, gather indices, experts)
    runs on device.
  - Device: router -> top-2 gates/masks; per-expert token lists are built
    with a cumsum matmul (positions) + an indirect-DMA scatter of token ids
    into a slot table; gpsimd ap_gather dispatches gelu(h) columns into
    per-expert contiguous slots; expert MLPs run dense on the gathered
    slots (bf16); a second ap_gather returns per-token expert outputs,
    scaled by the gates and accumulated.
  - Experts / shared / core matmuls in bf16 (fp32 PSUM accumulate), router/
    down/up in fp32r. Validated at ~2.4e-3 rel err vs fp32 reference.

Math rewrite (same as v1): unrouted-token "bias leak" folds into b2' and a
constant absorbed in the output bias; gate matrix W[t,e] is zero except
top-2. With zero biases these corrections vanish but are kept for
generality.
"""

import sys

import numpy as np

sys.path.insert(0, "/opt/trn_rl_repo")

B, T, D, L = 4, 2048, 2048, 1024
HID = [1024, 2048, 3072, 4096, 1024, 2048, 3072, 4096]
E = 8
TOP_K = 2
COST_LAMBDA = 1e-7
NCORES = 8
NTOK = B * T
TPC = NTOK // NCORES  # 1024 tokens per core
P = 128
KD = D // P   # 16
KL = L // P   # 8
NH = TPC // 512
NCH = TPC // P  # 8 token chunks of 128

# activation override for CoreSim validation (sim lacks Gelu)
_ACT = "gelu"
_DEBUG = False


def _gelu_np(v):
    from scipy.special import erf

    return 0.5 * v * (1.0 + erf(v / np.sqrt(2.0)))


def _route_host(x2d, rw, rb):
    """fp32 routing on host -> top2 indices [NTOK, 2]."""
    cost = np.array([2 * L * h for h in HID], np.float32)
    logits = x2d @ rw + rb - COST_LAMBDA * cost
    # top-2 by prob == top-2 by logit (softmax monotonic)
    top2 = np.argsort(-logits, axis=-1, kind="stable")[:, :2]
    return top2.astype(np.int64)


def _balance_tokens(top2):
    """Greedy token->core assignment equalizing per-core per-expert counts.

    Returns perm (token ids ordered core0..core7) and per-core counts [8, E].
    """
    gcnt = np.zeros(E, np.int64)
    for k in range(TOP_K):
        gcnt += np.bincount(top2[:, k], minlength=E)
    target = -(-gcnt // NCORES)  # ceil
    cnt = np.zeros((NCORES, E), np.int64)
    fill = np.zeros(NCORES, np.int64)
    assign = np.empty(NTOK, np.int64)
    slack = 4
    for t in range(NTOK):
        e1, e2 = top2[t, 0], top2[t, 1]
        best = -1
        for c in range(NCORES):
            if (
                fill[c] < TPC
                and cnt[c, e1] < target[e1] + slack
                and cnt[c, e2] < target[e2] + slack
            ):
                best = c
                break
        if best < 0:
            # fallback: least filled core
            best = int(np.argmin(np.where(fill < TPC, fill, 1 << 30)))
        assign[t] = best
        fill[best] += 1
        cnt[best, e1] += 1
        cnt[best, e2] += 1
    perm = np.argsort(assign, kind="stable")
    return perm, cnt


def _capacities(cnt):
    """Per-expert slot capacity (shared across cores), multiple of 16."""
    caps = []
    for e in range(E):
        c = int(cnt[:, e].max()) + 24
        c = ((c + 15) // 16) * 16
        c = max(c, 64)
        assert c <= 512, f"capacity {c} for expert {e} exceeds single-tile limit"
        caps.append(c)
    return caps


def _build_program(caps):
    import concourse.bass as bass
    from concourse import bacc
    import concourse.mybir as mybir
    import concourse.tile as tile
    from concourse.masks import make_identity

    f32 = mybir.dt.float32
    f32r = mybir.dt.float32r
    bf16 = mybir.dt.bfloat16
    i16 = mybir.dt.int16
    i32 = mybir.dt.int32
    AF = mybir.ActivationFunctionType
    ALU = mybir.AluOpType
    AX = mybir.AxisListType
    AF_ACT = AF.Gelu if _ACT == "gelu" else AF.Relu

    offs = [0]
    for c in caps:
        offs.append(offs[-1] + c)
    SCAP = offs[-1]
    assert SCAP % 16 == 0

    def r(ap):
        return ap

    nc = bacc.Bacc("TRN2", debug=False)

    # ---- DRAM I/O ----
    xt = nc.dram_tensor("xt", [D, TPC], f32r, kind="ExternalInput").ap()
    dw = nc.dram_tensor("dw", [D, L], f32r, kind="ExternalInput").ap()
    rw = nc.dram_tensor("rw", [D, E], f32r, kind="ExternalInput").ap()
    upw = nc.dram_tensor("upw", [L, D], f32r, kind="ExternalInput").ap()
    corw = nc.dram_tensor("corw", [D, D], bf16, kind="ExternalInput").ap()
    sw1 = nc.dram_tensor("sw1", [L, L], bf16, kind="ExternalInput").ap()
    sw2 = nc.dram_tensor("sw2", [L, L], bf16, kind="ExternalInput").ap()  # x0.1
    ew1 = [nc.dram_tensor(f"e{e}w1", [L, HID[e]], bf16, kind="ExternalInput").ap() for e in range(E)]
    ew2 = [nc.dram_tensor(f"e{e}w2", [HID[e], L], bf16, kind="ExternalInput").ap() for e in range(E)]
    dbpp = nc.dram_tensor("dbpp", [P, KL], f32, kind="ExternalInput").ap()
    sb1pp = nc.dram_tensor("sb1pp", [P, KL], f32, kind="ExternalInput").ap()
    b1pp = [nc.dram_tensor(f"b1pp{e}", [P, HID[e] // P], f32, kind="ExternalInput").ap() for e in range(E)]
    b2mat = nc.dram_tensor("b2mat", [E, L], f32r, kind="ExternalInput").ap()
    obias = nc.dram_tensor("obias", [1, D], f32r, kind="ExternalInput").ap()
    rbias = nc.dram_tensor("rbias", [1, E], f32r, kind="ExternalInput").ap()
    onesv = nc.dram_tensor("onesv", [1, 512], f32r, kind="ExternalInput").ap()
    ones8 = nc.dram_tensor("ones8", [8, 1], f32, kind="ExternalInput").ap()
    offcol = nc.dram_tensor("offcol", [8, 1], f32, kind="ExternalInput").ap()
    u3 = nc.dram_tensor("u3", [P, KL, TPC], bf16, kind="ExternalInput").ap()
    tokcol = nc.dram_tensor("tokcol", [P, NCH], i16, kind="ExternalInput").ap()
    out = nc.dram_tensor("out", [D, TPC], f32, kind="ExternalOutput").ap()
    if _DEBUG:
        dbg_sid = nc.dram_tensor("dbg_sid", [2, TPC], i16, kind="ExternalOutput").ap()
        dbg_idx = nc.dram_tensor("dbg_idx", [1, SCAP], i16, kind="ExternalOutput").ap()
        dbg_pos = nc.dram_tensor("dbg_pos", [E, TPC], f32, kind="ExternalOutput").ap()
    # internal DRAM scratch
    g2d = nc.dram_tensor("g2d_scratch", [2, TPC], bf16).ap()
    idxd = nc.dram_tensor("idxd_scratch", [1, SCAP], i16).ap()
    sidd = nc.dram_tensor("sidd_scratch", [2, TPC], i16).ap()

    with tile.TileContext(nc) as tc:
        import contextlib

        with contextlib.ExitStack() as ctx:
            const = ctx.enter_context(tc.tile_pool(name="const", bufs=1))
            gpool = ctx.enter_context(tc.tile_pool(name="gpool", bufs=1))
            psum = ctx.enter_context(tc.tile_pool(name="psum", bufs=2, space="PSUM"))
            gy_scope = contextlib.ExitStack()
            ggp = gy_scope.enter_context(tc.tile_pool(name="ggp", bufs=1))
            yap = gy_scope.enter_context(tc.tile_pool(name="yap", bufs=1))

            ident = const.tile([P, P], f32)
            make_identity(nc, ident)
            ones = const.tile([1, 512], f32r)
            nc.sync.dma_start(ones, onesv)
            ones8_sb = const.tile([8, 1], f32)
            nc.sync.dma_start(ones8_sb, ones8)
            offc_sb = const.tile([8, 1], f32)
            nc.sync.dma_start(offc_sb, offcol)

            rw_sb = const.tile([P, KD, E], f32r)
            nc.sync.dma_start(rw_sb, rw.rearrange("(ko ki) e -> ki ko e", ki=P))
            rb_sb = const.tile([1, E], f32r)
            nc.sync.dma_start(rb_sb, rbias)
            db_sb = const.tile([P, KL], f32)
            nc.sync.dma_start(db_sb, dbpp)
            sb1_sb = const.tile([P, KL], f32)
            nc.sync.dma_start(sb1_sb, sb1pp)
            b1_sb = []
            for e in range(E):
                t_ = const.tile([P, HID[e] // P], f32, tag=f"b1sb{e}")
                nc.sync.dma_start(t_, b1pp[e])
                b1_sb.append(t_)
            # routing state
            wt_sb = const.tile([E, TPC], f32r)    # gate matrix W[e, t]
            m12tm = const.tile([P, NCH, E], bf16)  # token-major mask sum

            y_fm = gpool.tile([P, KL, TPC], f32r)  # latent accumulator

            # short-lived pools (closed mid-build to free SBUF); LIFO: maskp
            # (closed after stage A2) must sit above gpp (closed after the
            # dispatch gathers).
            gp_scope = contextlib.ExitStack()
            gpp = gp_scope.enter_context(tc.tile_pool(name="gpp", bufs=1))
            # gelu(h) feature-major, interleaved KL-pairs for d=2 gathers:
            # g_pair[p, j, t, c] = gelu(h)[l=(2j+c)*128+p, t]
            g_pair = gpp.tile([P, KL // 2, TPC, 2], bf16)

            mask_scope = contextlib.ExitStack()
            maskp = mask_scope.enter_context(tc.tile_pool(name="maskp", bufs=1))
            m1_sb = maskp.tile([E, TPC], f32)     # top-1 one-hot
            m2_sb = maskp.tile([E, TPC], f32)     # top-2 one-hot
            g12b = maskp.tile([2, TPC], bf16)     # gates rows (bf16)

            xt3 = xt.rearrange("(ko ki) t -> ki ko t", ki=P)

            # ============ Stage A: router + down-projection ============
            with contextlib.ExitStack() as sab:
                xpool = sab.enter_context(tc.tile_pool(name="xpool", bufs=2))
                dwp = sab.enter_context(tc.tile_pool(name="dwp", bufs=2))
                rwork = sab.enter_context(tc.tile_pool(name="rwork", bufs=4))
                rpsum = sab.enter_context(tc.tile_pool(name="rpsum", bufs=2, space="PSUM"))
                tpsum = sab.enter_context(tc.tile_pool(name="tpsum", bufs=1, space="PSUM"))

                for quar in range(4):
                    ts_ = slice(quar * 256, (quar + 1) * 256)
                    xth = xpool.tile([P, KD, 256], f32r, tag="xth")
                    nc.sync.dma_start(xth, xt3[:, :, ts_])

                    # ---- router on this quarter's 2 token-chunks of 128 ----
                    for tj in range(2):
                        ch = quar * 2 + tj
                        t0 = ch * P
                        rp = rpsum.tile([P, E], f32, tag="rp")
                        for k in range(KD):
                            nc.tensor.matmul(
                                rp, r(xth[:, k, tj * 128:(tj + 1) * 128]), r(rw_sb[:, k, :]),
                                start=(k == 0), stop=False,
                            )
                        nc.tensor.matmul(rp, r(ones[:, :P]), r(rb_sb), start=False, stop=True)
                        nmax = rwork.tile([P, 1], f32, tag="nmax")
                        nc.vector.tensor_reduce(nmax, rp, axis=AX.X, op=ALU.max, negate=True)
                        pexp = rwork.tile([P, E], f32, tag="pexp")
                        nc.scalar.activation(pexp, rp, AF.Exp, bias=nmax)
                        ssum = rwork.tile([P, 1], f32, tag="ssum")
                        nc.vector.tensor_reduce(ssum, pexp, axis=AX.X, op=ALU.add)
                        rs = rwork.tile([P, 1], f32, tag="rs")
                        nc.vector.reciprocal(rs, ssum)
                        probs = rwork.tile([P, E], f32, tag="probs")
                        nc.vector.tensor_scalar_mul(probs, pexp, rs)
                        p1 = rwork.tile([P, 1], f32, tag="p1")
                        nc.vector.tensor_reduce(p1, probs, axis=AX.X, op=ALU.max)
                        mlt = rwork.tile([P, E], f32, tag="mlt")
                        nc.vector.tensor_scalar(mlt, probs, p1, None, op0=ALU.is_lt)
                        pz = rwork.tile([P, E], f32, tag="pz")
                        nc.vector.tensor_mul(pz, probs, mlt)
                        p2 = rwork.tile([P, 1], f32, tag="p2")
                        nc.vector.tensor_reduce(p2, pz, axis=AX.X, op=ALU.max)
                        dd = rwork.tile([P, 1], f32, tag="dd")
                        nc.vector.tensor_scalar(dd, p2, p1, None, op0=ALU.subtract)
                        s2 = rwork.tile([P, 1], f32, tag="s2")
                        nc.scalar.activation(s2, dd, AF.Sigmoid)
                        s1 = rwork.tile([P, 1], f32, tag="s1")
                        nc.vector.tensor_scalar(s1, s2, -1.0, 1.0, op0=ALU.mult, op1=ALU.add)
                        m1 = rwork.tile([P, E], f32, tag="m1")
                        nc.vector.tensor_scalar(m1, probs, p1, None, op0=ALU.is_ge)
                        m2 = rwork.tile([P, E], f32, tag="m2")
                        nc.vector.tensor_scalar(m2, pz, p2, None, op0=ALU.is_ge)
                        wc1 = rwork.tile([P, E], f32, tag="wc1")
                        nc.vector.tensor_scalar_mul(wc1, m1, s1)
                        wc = rwork.tile([P, E], f32, tag="wc")
                        nc.vector.tensor_scalar_mul(wc, m2, s2)
                        nc.vector.tensor_add(wc, wc, wc1)
                        # token-major mask sum (for cumsum matmul)
                        nc.vector.tensor_add(m12tm[:, ch, :], m1, m2)
                        # gates columns [128, 2]
                        gg = rwork.tile([P, 2], f32, tag="gg")
                        nc.vector.tensor_copy(gg[:, 0:1], s1)
                        nc.vector.tensor_copy(gg[:, 1:2], s2)
                        # transposes -> feature-major rows
                        tpw = tpsum.tile([E, P], f32, tag="tpw")
                        nc.tensor.transpose(tpw, wc, ident)
                        nc.vector.tensor_copy(wt_sb[:, t0:t0 + P], tpw)
                        tp1 = tpsum.tile([E, P], f32, tag="tp1")
                        nc.tensor.transpose(tp1, m1, ident)
                        nc.vector.tensor_copy(m1_sb[:, t0:t0 + P], tp1)
                        tp2 = tpsum.tile([E, P], f32, tag="tp2")
                        nc.tensor.transpose(tp2, m2, ident)
                        nc.vector.tensor_copy(m2_sb[:, t0:t0 + P], tp2)
                        tpg = tpsum.tile([2, P], f32, tag="tpg")
                        nc.tensor.transpose(tpg, gg, ident)
                        nc.vector.tensor_copy(g12b[:, t0:t0 + P], tpg)

                    # ---- down-projection for this quarter ----
                    for m in range(KL):
                        dsl = dwp.tile([P, KD, P], f32r, tag="dsl")
                        nc.sync.dma_start(
                            dsl, dw.rearrange("(ko ki) l -> ki ko l", ki=P)[:, :, m * P:(m + 1) * P]
                        )
                        hp = psum.tile([P, 512], f32, tag="a")
                        for k in range(KD):
                            nc.tensor.matmul(
                                hp[:, :256], r(dsl[:, k, :]), r(xth[:, k, :]),
                                start=(k == 0), stop=(k == KD - 1),
                            )
                        nc.scalar.activation(
                            g_pair[:, m // 2, ts_, m % 2], hp[:, :256], AF_ACT, bias=db_sb[:, m:m + 1]
                        )

                nc.sync.dma_start(g2d, g12b)

            # ============ Stage A2: slot indices ============
            idx_w = const.tile([P, SCAP // 16], i16)
            sidw = [
                const.tile([P, TPC // 16], i16, tag=f"sidw{k}", name=f"sidw{k}")
                for k in range(2)
            ]
            gates_b = const.tile([P, 2, TPC], bf16)

            with contextlib.ExitStack() as sa2:
                upool = sa2.enter_context(tc.tile_pool(name="upool", bufs=1))
                iwork = sa2.enter_context(tc.tile_pool(name="iwork", bufs=1))
                ipsum = sa2.enter_context(tc.tile_pool(name="ipsum", bufs=2, space="PSUM"))

                u3_sb = upool.tile([P, KL, TPC], bf16)
                nc.sync.dma_start(u3_sb, u3)
                tokc_sb = upool.tile([P, NCH], i16)
                nc.sync.dma_start(tokc_sb, tokcol)

                # POS[e, t] = inclusive cumsum over tokens of m12
                pos = upool.tile([E, TPC], f32)
                for half in range(NH):
                    ts_ = slice(half * 512, (half + 1) * 512)
                    pc = ipsum.tile([E, 512], f32, tag="pc")
                    for c in range(NCH):
                        nc.tensor.matmul(
                            pc, r(m12tm[:, c, :]), r(u3_sb[:, c, ts_]),
                            start=(c == 0), stop=(c == NCH - 1),
                        )
                    nc.vector.tensor_copy(pos[:, ts_], pc)

                # POSo = POS + (offset_e - 1)
                poso = upool.tile([E, TPC], f32)
                nc.vector.tensor_scalar(poso, pos, offc_sb, None, op0=ALU.add)

                # prefill idxd with zeros
                z16 = iwork.tile([1, SCAP], i16, tag="z16")
                nc.vector.memset(z16, 0)
                nc.sync.dma_start(idxd, z16)

                for k in range(2):
                    mk = m1_sb if k == 0 else m2_sb
                    prod = iwork.tile([E, TPC], f32, tag="prod")
                    nc.vector.tensor_mul(prod, mk, poso)
                    sid_f = iwork.tile([1, TPC], f32, tag="sidf")
                    for half in range(NH):
                        ts_ = slice(half * 512, (half + 1) * 512)
                        sp = ipsum.tile([1, 512], f32, tag="sp")
                        nc.tensor.matmul(sp, r(ones8_sb), r(prod[:, ts_]), start=True, stop=True)
                        nc.vector.tensor_copy(sid_f[:, ts_], sp)
                    sid16 = iwork.tile([1, TPC], i16, tag="sid16")
                    nc.vector.tensor_scalar(sid16, sid_f, 0.25, None, op0=ALU.add)
                    nc.sync.dma_start(sidd[k:k + 1, :], sid16)
                    # scatter token ids into slot table. The HW indirect DMA
                    # makes one descriptor per SBUF partition, so indices and
                    # payload must be columns: 8 scatters of 128 tokens each.
                    sidcol = iwork.tile([P, NCH], i32, tag=f"sidcol{k}", name=f"sidcol{k}")
                    for c in range(NCH):
                        tcp = ipsum.tile([P, 1], f32, tag="tcp")
                        nc.tensor.transpose(tcp, sid_f[:, c * P:(c + 1) * P], ident[:1, :1])
                        nc.vector.tensor_scalar(sidcol[:, c:c + 1], tcp, 0.25, None, op0=ALU.add)
                    for c in range(NCH):
                        nc.gpsimd.indirect_dma_start(
                            out=idxd,
                            out_offset=bass.IndirectOffsetOnAxis(ap=sidcol[:, c:c + 1], axis=1),
                            in_=tokc_sb[:, c:c + 1],
                            in_offset=None,
                        )

                if _DEBUG:
                    dbt = iwork.tile([1, SCAP], i16, tag="dbt")
                    nc.sync.dma_start(dbt, idxd)
                    nc.sync.dma_start(dbg_idx, dbt)
                    dbs = iwork.tile([2, TPC], i16, tag="dbs")
                    nc.sync.dma_start(dbs, sidd)
                    nc.sync.dma_start(dbg_sid, dbs)
                    nc.sync.dma_start(dbg_pos, pos)

                # wrapped-16 loads (replicated to all 8 gpsimd cores)
                idxd_w = idxd.rearrange("o (f p) -> (o p) f", p=16)
                for grp in range(8):
                    nc.sync.dma_start(idx_w[grp * 16:(grp + 1) * 16, :], idxd_w)
                for k in range(2):
                    sdw = sidd[k:k + 1, :].rearrange("o (f p) -> (o p) f", p=16)
                    for grp in range(8):
                        nc.sync.dma_start(sidw[k][grp * 16:(grp + 1) * 16, :], sdw)
                nc.sync.dma_start(gates_b, g2d.partition_broadcast(P))

            mask_scope.close()

            # ============ Stage B: b2' correction + shared branch ============
            with contextlib.ExitStack() as sb:
                swp = sb.enter_context(tc.tile_pool(name="swp", bufs=1))
                shg = sb.enter_context(tc.tile_pool(name="shg", bufs=1))

                b2_sb = swp.tile([E, L], f32r)
                nc.sync.dma_start(b2_sb, b2mat)

                for m in range(KL):
                    for half in range(NH):
                        ts_ = slice(half * 512, (half + 1) * 512)
                        yp = psum.tile([P, 512], f32, tag="a")
                        nc.tensor.matmul(
                            yp, r(b2_sb[:, m * P:(m + 1) * P]), r(wt_sb[:, ts_]),
                            start=True, stop=True,
                        )
                        nc.vector.tensor_copy(y_fm[:, m, ts_], yp)

                sw1_sb = swp.tile([P, KL, L], bf16, tag="sw1")
                nc.sync.dma_start(sw1_sb, sw1.rearrange("(ko ki) l -> ki ko l", ki=P))
                sw2_sb = swp.tile([P, KL, L], bf16, tag="sw2")
                nc.sync.dma_start(sw2_sb, sw2.rearrange("(ko ki) l -> ki ko l", ki=P))
                gsh = shg.tile([P, KL, TPC], bf16)
                for m in range(KL):
                    for half in range(NH):
                        ts_ = slice(half * 512, (half + 1) * 512)
                        ap_ = psum.tile([P, 512], f32, tag="a")
                        for k in range(KL):
                            nc.tensor.matmul(
                                ap_, r(sw1_sb[:, k, m * P:(m + 1) * P]),
                                r(g_pair[:, k // 2, ts_, k % 2]),
                                start=(k == 0), stop=(k == KL - 1),
                            )
                        nc.scalar.activation(gsh[:, m, ts_], ap_, AF_ACT, bias=sb1_sb[:, m:m + 1])
                for m in range(KL):
                    for half in range(NH):
                        ts_ = slice(half * 512, (half + 1) * 512)
                        ap_ = psum.tile([P, 512], f32, tag="a")
                        for k in range(KL):
                            nc.tensor.matmul(
                                ap_, r(sw2_sb[:, k, m * P:(m + 1) * P]), r(gsh[:, k, ts_]),
                                start=(k == 0), stop=(k == KL - 1),
                            )
                        nc.vector.tensor_add(y_fm[:, m, ts_], y_fm[:, m, ts_], ap_)

            # ============ Stage C: gather + experts ============
            A1GSZ = max((HID[e] // P) * caps[e] for e in range(E))
            A1GSZ = ((A1GSZ + 63) // 64) * 64
            W2ESZ = (max(HID) // P) * P  # one 128-col slice of w2, all h rows
            g_gath = ggp.tile([P, KL // 2, SCAP, 2], bf16)
            y_all = yap.tile([P, KL // 2, SCAP, 2], bf16)

            for j in range(KL // 2):
                nc.gpsimd.ap_gather(
                    g_gath[:, j, :, :], g_pair[:, j, :, :], idx_w,
                    channels=P, num_elems=TPC, d=2, num_idxs=SCAP,
                )

            gp_scope.close()

            with contextlib.ExitStack() as sc:
                wep = sc.enter_context(tc.tile_pool(name="wep", bufs=2))
                apool = sc.enter_context(tc.tile_pool(name="apool", bufs=2))
                psA = sc.enter_context(tc.tile_pool(name="psA", bufs=2, space="PSUM"))
                psB = sc.enter_context(tc.tile_pool(name="psB", bufs=2, space="PSUM"))

                for e in range(E):
                    h_e = HID[e]
                    nhc = h_e // P
                    ce = caps[e]
                    o0 = offs[e]
                    a1g = apool.tile([P, A1GSZ], bf16, tag="a1g")
                    # ---- mm1: a1 = gelu(w1.T @ g_gath + b1), streamed in 512-row groups
                    for gi in range(h_e // 512):
                        w1s = wep.tile([P, KL, 512], bf16, tag="w1s")
                        nc.sync.dma_start(
                            w1s,
                            ew1[e].rearrange("(ko ki) h -> ki ko h", ki=P)[:, :, gi * 512:(gi + 1) * 512],
                        )
                        for hc4 in range(4):
                            hc = gi * 4 + hc4
                            ap_ = psA.tile([P, 512], f32, tag="pa")
                            for k in range(KL):
                                nc.tensor.matmul(
                                    ap_[:, :ce], r(w1s[:, k, hc4 * P:(hc4 + 1) * P]),
                                    r(g_gath[:, k // 2, o0:o0 + ce, k % 2]),
                                    start=(k == 0), stop=(k == KL - 1),
                                )
                            nc.scalar.activation(
                                a1g[:, hc * ce:(hc + 1) * ce], ap_[:, :ce], AF_ACT,
                                bias=b1_sb[e][:, hc:hc + 1],
                            )
                    # ---- mm2: y_e = w2.T @ a1, l in eighths of 128
                    for m in range(KL):
                        w2e = wep.tile([P, W2ESZ], bf16, tag="w2e")
                        nc.sync.dma_start(
                            w2e[:, :nhc * P].rearrange("p (ko l) -> p ko l", ko=nhc),
                            ew2[e].rearrange("(ko ki) l -> ki ko l", ki=P)[:, :, m * P:(m + 1) * P],
                        )
                        yp = psB.tile([P, 512], f32, tag="pb")
                        for hc in range(nhc):
                            nc.tensor.matmul(
                                yp[:, :ce],
                                r(w2e[:, hc * P:(hc + 1) * P]),
                                r(a1g[:, hc * ce:(hc + 1) * ce]),
                                start=(hc == 0), stop=(hc == nhc - 1),
                            )
                        nc.vector.tensor_copy(y_all[:, m // 2, o0:o0 + ce, m % 2], yp[:, :ce])

                # ============ Stage D: gather-back + gate scale ============
                with contextlib.ExitStack() as sd:
                    dpool = sd.enter_context(tc.tile_pool(name="dpool", bufs=2))
                    for k in range(2):
                        for j in range(KL // 2):
                            yk = dpool.tile([P, TPC, 2], bf16, tag="yk")
                            nc.gpsimd.ap_gather(
                                yk, y_all[:, j, :, :], sidw[k],
                                channels=P, num_elems=SCAP, d=2, num_idxs=TPC,
                            )
                            for c in range(2):
                                m = 2 * j + c
                                for half in range(NH):
                                    ts_ = slice(half * 512, (half + 1) * 512)
                                    tmp = dpool.tile([P, 512], f32, tag="tmp")
                                    nc.vector.tensor_mul(tmp, yk[:, ts_, c], gates_b[:, k, ts_])
                                    nc.vector.tensor_add(y_fm[:, m, ts_], y_fm[:, m, ts_], tmp)

            gy_scope.close()

            # ============ Stage E: up-projection + core branch ============
            with contextlib.ExitStack() as se:
                gxp = se.enter_context(tc.tile_pool(name="gxp", bufs=1))
                stg = se.enter_context(tc.tile_pool(name="stg", bufs=2))
                wup = se.enter_context(tc.tile_pool(name="wup", bufs=2))
                otp = se.enter_context(tc.tile_pool(name="otp", bufs=3))

                ob_sb = gxp.tile([1, D], f32r)
                nc.sync.dma_start(ob_sb, obias)

                gx = gxp.tile([P, KD, TPC], bf16)
                for quar in range(4):
                    ts_ = slice(quar * 256, (quar + 1) * 256)
                    st_ = stg.tile([P, KD, 256], f32r, tag="st")
                    nc.sync.dma_start(st_, xt3[:, :, ts_])
                    for k in range(KD):
                        nc.scalar.activation(gx[:, k, ts_], st_[:, k, :], AF_ACT)

                for m in range(KD):
                    ms = slice(m * P, (m + 1) * P)
                    usl = wup.tile([P, KL, P], f32r, tag="usl")
                    nc.sync.dma_start(usl, upw.rearrange("(ko ki) d -> ki ko d", ki=P)[:, :, ms])
                    csl = wup.tile([P, KD, P], bf16, tag="csl")
                    nc.sync.dma_start(csl, corw.rearrange("(ko ki) d -> ki ko d", ki=P)[:, :, ms])
                    for half in range(NH):
                        ts_ = slice(half * 512, (half + 1) * 512)
                        op_ = psum.tile([P, 512], f32, tag="a")
                        for k in range(KL):
                            nc.tensor.matmul(op_, r(usl[:, k, :]), r(y_fm[:, k, ts_]), start=(k == 0), stop=False)
                        for k in range(KD):
                            nc.tensor.matmul(op_, r(csl[:, k, :]), r(gx[:, k, ts_]), start=False, stop=False)
                        nc.tensor.matmul(op_, r(ob_sb[:1, ms]), r(ones[:1, :512]), start=False, stop=True)
                        ot = otp.tile([P, 512], f32, tag="ot")
                        nc.vector.tensor_copy(ot, op_)
                        nc.sync.dma_start(out[ms, ts_], ot)

    nc.finalize()
    return nc


def _prep_host(inputs):
    """Host-side preprocessing: routing for capacities, balancing, weight prep.

    Returns (nc_builder_args, in_maps, perm).
    """
    import ml_dtypes

    bfl = ml_dtypes.bfloat16

    inp = {k: np.ascontiguousarray(np.asarray(v, dtype=np.float32)) for k, v in inputs.items()}
    x = inp["x"].reshape(NTOK, D)

    top2 = _route_host(x, inp["router_w"], inp["router_b"])
    perm, cnt = _balance_tokens(top2)
    caps = _capacities(cnt)

    cost = np.array([2 * L * h for h in HID], np.float32)
    rbias = (inp["router_b"] - COST_LAMBDA * cost).reshape(1, E)
    c = [_gelu_np(inp[f"e{e}_b1"]) @ inp[f"e{e}_w2"] + inp[f"e{e}_b2"] for e in range(E)]
    b2mat = np.stack([inp[f"e{e}_b2"] - c[e] for e in range(E)], axis=0)
    const_l = np.sum(c, axis=0) + 0.1 * inp["shared_b2"]
    obias = (inp["up_b"] + const_l @ inp["up_w"] + inp["core_b"]).reshape(1, D)

    offs = np.concatenate([[0], np.cumsum(caps)])
    offcol = (offs[:E].astype(np.float32) - 1.0).reshape(E, 1)

    # U triangular, laid out [ki, ko, t] bf16: 1 if ko*128+ki <= t
    tt = np.arange(TPC)
    rowidx = (np.arange(P)[:, None, None] + 128 * np.arange(KL)[None, :, None])
    u3 = (rowidx <= tt[None, None, :]).astype(bfl)

    common = {
        "dw": inp["down_w"],
        "rw": inp["router_w"],
        "upw": inp["up_w"],
        "corw": inp["core_w"].astype(bfl),
        "sw1": inp["shared_w1"].astype(bfl),
        "sw2": np.ascontiguousarray(0.1 * inp["shared_w2"]).astype(bfl),
        "dbpp": np.ascontiguousarray(inp["down_b"].reshape(KL, P).T),
        "sb1pp": np.ascontiguousarray(inp["shared_b1"].reshape(KL, P).T),
        "b2mat": np.ascontiguousarray(b2mat),
        "obias": np.ascontiguousarray(obias),
        "rbias": np.ascontiguousarray(rbias),
        "onesv": np.ones((1, 512), np.float32),
        "ones8": np.ones((8, 1), np.float32),
        "offcol": np.ascontiguousarray(offcol),
        "u3": np.ascontiguousarray(u3),
        "tokcol": np.ascontiguousarray(
            np.arange(TPC, dtype=np.int16).reshape(NCH, P).T
        ),
    }
    for e in range(E):
        common[f"e{e}w1"] = inp[f"e{e}_w1"].astype(bfl)
        common[f"e{e}w2"] = inp[f"e{e}_w2"].astype(bfl)
        common[f"b1pp{e}"] = np.ascontiguousarray(inp[f"e{e}_b1"].reshape(HID[e] // P, P).T)

    xp = x[perm]
    in_maps = []
    for cidx in range(NCORES):
        m = dict(common)
        m["xt"] = np.ascontiguousarray(xp[cidx * TPC:(cidx + 1) * TPC].T)
        in_maps.append(m)
    return caps, in_maps, perm


def kernel(**inputs):
    from concourse.bass_utils import run_bass_kernel_spmd

    caps, in_maps, perm = _prep_host(inputs)
    nc = _build_program(caps)
    res = run_bass_kernel_spmd(nc, in_maps, list(range(NCORES)))

    full = np.empty((NTOK, D), np.float32)
    for cidx in range(NCORES):
        full[perm[cidx * TPC:(cidx + 1) * TPC]] = res.results[cidx]["out"].T
    return full.reshape(B, T, D)
